# revision 1
# baseline (speedup 1.0000x reference)
"""Trainium2 Bass kernel for nn_GCIQEValue (MLP + IQE head), 8-core data parallel.

Math (validated vs reference):
  phi(x) = LN-MLP: 3x [matmul+bias -> tanh-gelu -> LayerNorm(affine folded into
  next W on host)] then final matmul+bias.
  IQE per row, per 32-dim component c with x = phi_s[c], y = phi_g[c]:
    y' = max(x, y)                      (interval [x_i, max(x_i,y_i)])
    u = sort(x), v = sort(y')           (independent keys-only sorts: the
                                         union-measure depends only on the
                                         multisets of starts/ends)
    comp_c = sum(v) - u_0 - sum_{i>=1} max(u_i, v_{i-1})
  out = sig(alpha) * mean_c(comp) + (1 - sig(alpha)) * max_c(comp)

Structure: 7-stage software pipeline (For_i_pipelined) over 128-row tiles:
  S0 load | S1 L0 | S2 L1 | S3 L2 | S4 L3+ymax | S5 sort p0-7 | S6 sort p8-14+post
LN statistics ride ACT accum_out (sum of gelu / sum of squares); the bitonic
sort runs on DVE as strided min/max tensor_tensor pairs.
"""

import numpy as np

B = 131072
OBS = 64
H = 512
NCOMP = 16
DPC = 32
NCORES = 8
P = 128
LN_EPS = 1e-6

_CACHE = {}

# bitonic schedule for 32-wide ascending sort: 15 passes
_SCHED = [("pair", 0, 0)]
for _L in (4, 8, 16, 32):
    _SCHED.append(("flip", _L, 0))
    _d = _L // 4
    while _d >= 1:
        _SCHED.append(("shift", _L, _d))
        _d //= 2


# ---------------------------------------------------------------- device kernel
def build_nc(rows_per_core=B // NCORES, unroll=4, gelu="hw", repeats=1,
             stage_bufs=None, mlp_bufs=3, psum_bufs=4, split_pass=7,
             n_passes=15, sort_chunks=1, n_layers=3, ln_lite=False,
             rsqrt_newton=True, bias_mode="pe", split_l0=True, hints=False):
    """Build the Bass (Bacc) module for one core processing rows_per_core rows."""
    import concourse.bass as bass
    import concourse.mybir as mybir
    import concourse.tile as tile
    from concourse import bacc
    from concourse.masks import make_identity

    fp32 = mybir.dt.float32
    AT = mybir.ActivationFunctionType
    OP = mybir.AluOpType

    nt = rows_per_core // P
    assert rows_per_core % P == 0
    if stage_bufs is None:
        stage_bufs = unroll

    nc = bacc.Bacc("TRN2", target_bir_lowering=False, debug=False)

    obs = nc.declare_dram_parameter("observations", [rows_per_core, OBS], fp32,
                                    isOutput=False)
    gls = nc.declare_dram_parameter("goals", [rows_per_core, OBS], fp32,
                                    isOutput=False)
    w0d = nc.declare_dram_parameter("w0", [OBS, H], fp32, isOutput=False)
    w1d = nc.declare_dram_parameter("w1", [H, H], fp32, isOutput=False)
    w2d = nc.declare_dram_parameter("w2", [H, H], fp32, isOutput=False)
    w3d = nc.declare_dram_parameter("w3", [H, H], fp32, isOutput=False)
    bsd = nc.declare_dram_parameter("bs", [4, H], fp32, isOutput=False)
    avd = nc.declare_dram_parameter("avec", [P, 2], fp32, isOutput=False)
    out = nc.declare_dram_parameter("out", [rows_per_core], fp32, isOutput=True)

    obs_v = obs[:].rearrange("(n p) f -> n p f", p=P)
    gls_v = gls[:].rearrange("(n p) f -> n p f", p=P)
    out_v = out[:].rearrange("(n p) -> n p", p=P)

    gelu_f = AT.Gelu_apprx_tanh if gelu == "hw" else AT.Identity

    with tile.TileContext(nc) as tc:
        with (
            tc.tile_pool(name="const", bufs=1) as cpool,
            tc.tile_pool(name="mlp", bufs=mlp_bufs) as mp,
            tc.tile_pool(name="srt", bufs=mlp_bufs) as sp,
            tc.tile_pool(name="pipe", bufs=1) as pipe_pool,
            tc.tile_pool(name="ps", bufs=psum_bufs, space="PSUM") as pp,
            tc.tile_pool(name="pst", bufs=8 - psum_bufs, space="PSUM") as ppt,
        ):
            # ---- constants
            w0 = cpool.tile([OBS, H], fp32)
            nc.sync.dma_start(out=w0, in_=w0d[:])
            wl = []
            for wd, nm in ((w1d, "w1"), (w2d, "w2"), (w3d, "w3")):
                t = cpool.tile([P, 4, H], fp32, tag=nm)
                nc.sync.dma_start(out=t, in_=wd[:].rearrange("(c p) n -> p c n", p=P))
                wl.append(t)
            bsc = cpool.tile([1, 4, H], fp32)
            nc.sync.dma_start(out=bsc, in_=bsd[:].rearrange("(o c) n -> o c n", o=1))
            if bias_mode != "pe":
                bsb = cpool.tile([P, 4, H], fp32)
                nc.sync.dma_start(
                    out=bsb,
                    in_=bass.AP(tensor=bsd[:].tensor, offset=0,
                                ap=[[0, P]] + list(bsd[:].ap)))
            avec = cpool.tile([P, 2], fp32)
            nc.sync.dma_start(out=avec, in_=avd[:])
            ident = cpool.tile([P, P], fp32)
            make_identity(nc, ident)
            ones = cpool.tile([1, P], fp32)
            nc.vector.memset(ones, 1.0)
            epst = cpool.tile([P, 1], fp32)
            nc.vector.memset(epst, LN_EPS)

            def matmul_from(t_sb, li):
                """t_sb [128, F_in] row-major -> pz PSUM [128, 512] for layer li
                (li = 0 uses w0/64-wide input, else wl[li-1])."""
                pz = pp.tile([P, H], fp32, tag="pz")
                start = True
                if bias_mode != "pe":
                    eng = nc.scalar if bias_mode == "act" else nc.vector
                    if bias_mode == "act":
                        eng.copy(pz, bsb[:, li, :])
                    else:
                        eng.tensor_copy(pz, bsb[:, li, :])
                    start = False
                if li == 0:
                    pTf = ppt.tile([P, H], fp32, tag="pT")
                    nc.tensor.transpose(pTf[0:OBS, 0:P], t_sb, ident)
                    xT = mp.tile([OBS, P], fp32, tag="xT")
                    nc.scalar.copy(xT, pTf[0:OBS, 0:P])
                    nc.tensor.matmul(pz, xT, w0, start=start,
                                     stop=(bias_mode != "pe"))
                else:
                    pTf = ppt.tile([P, H], fp32, tag="pT")
                    for k in range(4):
                        nc.tensor.transpose(pTf[:, k * P:(k + 1) * P],
                                            t_sb[:, k * P:(k + 1) * P], ident)
                    tT = mp.tile([P, 4, P], fp32, tag="tT")
                    nc.scalar.copy(tT, pTf)
                    for k in range(4):
                        nc.tensor.matmul(pz, tT[:, k, :], wl[li - 1][:, k, :],
                                         start=(start and k == 0),
                                         stop=(bias_mode != "pe" and k == 3))
                if bias_mode == "pe":
                    nc.tensor.matmul(pz, ones, bsc[:, li, :], start=False,
                                     stop=True)
                return pz

            def gelu_ln(pz, t_out):
                """pz PSUM -> t_out SBUF: LayerNorm(gelu(pz)) via ACT stats."""
                if ln_lite:
                    nc.scalar.activation(t_out, pz, gelu_f)
                    return
                g = mp.tile([P, H], fp32, tag="g")
                sums = mp.tile([P, 2], fp32, tag="sums")
                nc.scalar.activation(g, pz, gelu_f, accum_out=sums[:, 0:1])
                gsq = mp.tile([P, H], fp32, tag="gsq")
                nc.scalar.activation(gsq, g, AT.Square, accum_out=sums[:, 1:2])
                mv2 = mp.tile([P, 2], fp32, tag="mv2")
                nc.vector.tensor_scalar_mul(mv2, sums, 1.0 / H)
                msq = mp.tile([P, 1], fp32, tag="msq")
                nc.vector.tensor_tensor(out=msq, in0=mv2[:, 0:1],
                                        in1=mv2[:, 0:1], op=OP.mult)
                varb = mp.tile([P, 1], fp32, tag="varb")
                nc.vector.tensor_tensor(out=varb, in0=mv2[:, 1:2], in1=msq,
                                        op=OP.subtract)
                nc.vector.tensor_scalar_add(varb, varb, LN_EPS)
                rstd = mp.tile([P, 1], fp32, tag="rstd")
                if rsqrt_newton:
                    # rsqrt without the ACT Sqrt table set: quake seed on DVE
                    # int ALU + 3 Newton iterations (rel err ~1e-7).
                    i32 = mybir.dt.int32
                    yi = mp.tile([P, 1], i32, tag="yi")
                    nc.vector.tensor_scalar(
                        out=yi, in0=varb.bitcast(i32), scalar1=1,
                        scalar2=None, op0=OP.logical_shift_right)
                    nc.vector.tensor_scalar(
                        out=yi, in0=yi, scalar1=-1, scalar2=0x5F3759DF,
                        op0=OP.mult, op1=OP.add)
                    y = yi.bitcast(fp32)
                    t1 = mp.tile([P, 1], fp32, tag="nt1")
                    for _ in range(3):
                        nc.vector.tensor_tensor(out=t1, in0=varb, in1=y,
                                                op=OP.mult)
                        nc.vector.tensor_tensor(out=t1, in0=t1, in1=y,
                                                op=OP.mult)
                        nc.vector.tensor_scalar(out=t1, in0=t1, scalar1=-0.5,
                                                scalar2=1.5, op0=OP.mult,
                                                op1=OP.add)
                        nc.vector.tensor_tensor(out=y, in0=y, in1=t1,
                                                op=OP.mult)
                    rstd = y
                else:
                    std = mp.tile([P, 1], fp32, tag="std")
                    nc.scalar.activation(std, varb, AT.Sqrt)
                    nc.vector.reciprocal(rstd, std)
                nmr = mp.tile([P, 1], fp32, tag="nmr")
                nc.vector.scalar_tensor_tensor(out=nmr, in0=mv2[:, 0:1],
                                               scalar=-1.0, in1=rstd,
                                               op0=OP.mult, op1=OP.mult)
                nc.scalar.activation(t_out, g, AT.Identity, bias=nmr, scale=rstd)

            def emit_sort_pass(p_idx, src_x, src_y, dst):
                """Emit bitonic pass p_idx. Pass 0 reads (src_x, src_y) pair
                tensors; later passes read src_x as the full [P,1024] buffer.
                sort_chunks splits each instruction along the group dim to
                amortize the DVE post-op DRAIN (cost ~ 2*dur - const)."""
                kind, L, d = _SCHED[p_idx]
                V = nc.vector
                C = sort_chunks

                def ch(view, c):
                    n = view.shape[1]
                    return view[:, c * n // C:(c + 1) * n // C]

                if kind == "pair":
                    for src, off in ((src_x, 0), (src_y, H)):
                        s = src.rearrange("p (g e) -> p g e", e=DPC)
                        o = dst[:, off:off + H].rearrange("p (g e) -> p g e",
                                                          e=DPC)
                        for c in range(C):
                            sc, oc = ch(s, c), ch(o, c)
                            V.tensor_tensor(out=oc[:, :, 0::2],
                                            in0=sc[:, :, 0::2],
                                            in1=sc[:, :, 1::2], op=OP.min)
                            V.tensor_tensor(out=oc[:, :, 1::2],
                                            in0=sc[:, :, 0::2],
                                            in1=sc[:, :, 1::2], op=OP.max)
                elif kind == "flip":
                    half = L // 2
                    s = src_x.rearrange("p (b e) -> p b e", e=L)
                    o = dst.rearrange("p (b e) -> p b e", e=L)
                    for c in range(C):
                        sc, oc = ch(s, c), ch(o, c)
                        V.tensor_tensor(out=oc[:, :, 0:half],
                                        in0=sc[:, :, 0:half],
                                        in1=sc[:, :, L - 1:half - 1:-1],
                                        op=OP.min)
                        V.tensor_tensor(out=oc[:, :, half:L],
                                        in0=sc[:, :, half:L],
                                        in1=sc[:, :, half - 1::-1], op=OP.max)
                else:
                    s = src_x.rearrange("p (c e) -> p c e", e=2 * d)
                    o = dst.rearrange("p (c e) -> p c e", e=2 * d)
                    for c in range(C):
                        sc, oc = ch(s, c), ch(o, c)
                        V.tensor_tensor(out=oc[:, :, 0:d], in0=sc[:, :, 0:d],
                                        in1=sc[:, :, d:2 * d], op=OP.min)
                        V.tensor_tensor(out=oc[:, :, d:2 * d],
                                        in0=sc[:, :, 0:d],
                                        in1=sc[:, :, d:2 * d], op=OP.max)

            # ---------------- pipeline stages
            def st_load(pipe, iv):
                xt = pipe.intermediate_tile([P, OBS], fp32, name="xt")
                gt = pipe.intermediate_tile([P, OBS], fp32, name="gt")
                nc.sync.dma_start(out=xt, in_=obs_v[iv])
                nc.sync.dma_start(out=gt, in_=gls_v[iv])
                return (xt, gt)

            def mk_layer(li):
                def st(pipe, iv, prev):
                    to, tg = prev
                    if n_layers < 3:  # ablation: copy-through this mid layer
                        oo = pipe.intermediate_tile([P, H], fp32, name=f"to{li}")
                        og = pipe.intermediate_tile([P, H], fp32, name=f"tg{li}")
                        nc.scalar.copy(oo, to)
                        nc.scalar.copy(og, tg)
                        return (oo, og)
                    oo = pipe.intermediate_tile([P, H], fp32, name=f"to{li}")
                    og = pipe.intermediate_tile([P, H], fp32, name=f"tg{li}")
                    gelu_ln(matmul_from(to, li), oo)
                    gelu_ln(matmul_from(tg, li), og)
                    return (oo, og)
                return st

            def st_l3(pipe, iv, prev):
                to, tg = prev
                phis = pipe.intermediate_tile([P, H], fp32, name="phis")
                pz = matmul_from(to, 3)
                nc.scalar.copy(phis, pz)
                pzg = matmul_from(tg, 3)
                ypr = pipe.intermediate_tile([P, H], fp32, name="ypr")
                nc.vector.tensor_tensor(out=ypr, in0=phis, in1=pzg, op=OP.max)
                return (phis, ypr)

            def st_sort_a(pipe, iv, prev):
                phis, ypr = prev
                bufA = pipe.intermediate_tile([P, 2 * H], fp32, name="bufA")
                bufB = pipe.intermediate_tile([P, 2 * H], fp32, name="bufB")
                emit_sort_pass(0, phis, ypr, bufA)
                cur, nxt = bufA, bufB
                for pidx in range(1, split_pass):
                    if pidx < n_passes:
                        emit_sort_pass(pidx, cur, None, nxt)
                    cur, nxt = nxt, cur
                return (bufA, bufB)

            def st_sort_b(pipe, iv, prev):
                bufA, bufB = prev
                cur, nxt = (bufB, bufA) if split_pass % 2 == 0 else (bufA, bufB)
                for pidx in range(split_pass, 15):
                    if pidx < n_passes:
                        emit_sort_pass(pidx, cur, None, nxt)
                    cur, nxt = nxt, cur
                fin = cur  # pass 14 (even) -> bufA when split parity works out
                fv = fin.rearrange("p (h g e) -> p h g e", h=2, e=DPC)
                # coupling: u[i] <- max(u[i], v[i-1]) for i>=1, in place
                nc.vector.tensor_tensor(out=fv[:, 0, :, 1:DPC],
                                        in0=fv[:, 0, :, 1:DPC],
                                        in1=fv[:, 1, :, 0:DPC - 1], op=OP.max)
                red = sp.tile([P, 2, NCOMP], fp32, tag="red")
                nc.vector.tensor_reduce(out=red, in_=fv,
                                        axis=mybir.AxisListType.X, op=OP.add)
                comp = sp.tile([P, NCOMP], fp32, tag="comp")
                nc.vector.tensor_tensor(out=comp, in0=red[:, 1, :],
                                        in1=red[:, 0, :], op=OP.subtract)
                cs = sp.tile([P, 1], fp32, tag="cs")
                nc.vector.tensor_reduce(out=cs, in_=comp,
                                        axis=mybir.AxisListType.X, op=OP.add)
                cm = sp.tile([P, 1], fp32, tag="cm")
                nc.vector.tensor_reduce(out=cm, in_=comp,
                                        axis=mybir.AxisListType.X, op=OP.max)
                res = sp.tile([P, 1], fp32, tag="res")
                nc.vector.tensor_scalar(out=res, in0=cs, scalar1=avec[:, 0:1],
                                        scalar2=None, op0=OP.mult)
                nc.vector.scalar_tensor_tensor(out=res, in0=cm,
                                               scalar=avec[:, 1:2], in1=res,
                                               op0=OP.mult, op1=OP.add)
                nc.sync.dma_start(out=out_v[iv], in_=res[:, 0:1])

            def st_l01(pipe, iv, prev):
                xt, gt = prev
                t0o = mp.tile([P, H], fp32, tag="t0o")
                t0g = mp.tile([P, H], fp32, tag="t0g")
                gelu_ln(matmul_from(xt, 0), t0o)
                gelu_ln(matmul_from(gt, 0), t0g)
                oo = pipe.intermediate_tile([P, H], fp32, name="to1")
                og = pipe.intermediate_tile([P, H], fp32, name="tg1")
                gelu_ln(matmul_from(t0o, 1), oo)
                gelu_ln(matmul_from(t0g, 1), og)
                return (oo, og)

            def st_l0(pipe, iv, prev):
                xt, gt = prev
                oo = pipe.intermediate_tile([P, H], fp32, name="to0")
                og = pipe.intermediate_tile([P, H], fp32, name="tg0")
                gelu_ln(matmul_from(xt, 0), oo)
                gelu_ln(matmul_from(gt, 0), og)
                return (oo, og)

            if split_l0:
                stages = [st_load, st_l0, mk_layer(1), mk_layer(2), st_l3,
                          st_sort_a, st_sort_b]
            else:
                stages = [st_load, st_l01, mk_layer(2), st_l3,
                          st_sort_a, st_sort_b]

            def run_pipe():
                he = (mybir.EngineType.PE, mybir.EngineType.DVE,
                      mybir.EngineType.Activation, mybir.EngineType.SP,
                      mybir.EngineType.Pool) if hints else ()
                tc.For_i_pipelined(stages, 0, nt, 1, pool=pipe_pool,
                                   unroll=unroll, staged_num_bufs=stage_bufs,
                                   hint_engines=he)

            if repeats == 1:
                run_pipe()
            else:
                with tc.For_i(0, repeats, 1):
                    run_pipe()

    nc.finalize()
    return nc


# ---------------------------------------------------------------- host wrapper
def _prep_host(inputs):
    """Fold LN affine params into the following layer's weights; build avec."""
    f32 = np.float32
    W0 = np.asarray(inputs["W0"], f32)
    b0 = np.asarray(inputs["b0"], f32)
    w, b = [W0], [b0]
    for i in (0, 1, 2):
        s = np.asarray(inputs[f"ln{i}_s"], f32)
        t = np.asarray(inputs[f"ln{i}_b"], f32)
        Wn = np.asarray(inputs[("W1", "W2", "W3")[i]], f32)
        bn = np.asarray(inputs[("b1", "b2", "b3")[i]], f32)
        w.append(s[:, None] * Wn)
        b.append(bn + t @ Wn)
    bs = np.stack(b, 0)  # [4, 512]
    alpha = float(np.asarray(inputs["alpha"]))
    a = 1.0 / (1.0 + np.exp(-alpha))
    avec = np.empty((P, 2), f32)
    avec[:, 0] = a / NCOMP
    avec[:, 1] = 1.0 - a
    return w[0], w[1], w[2], w[3], bs.astype(f32), avec


def _probe_devices():
    """Poke every core with a tiny op; retries to shake off a stale
    NRT_EXEC_UNIT_UNRECOVERABLE state left by a previous process."""
    import jax
    import jax.numpy as jnp

    for attempt in range(3):
        try:
            for d in jax.devices()[:NCORES]:
                jnp.zeros((1,), jnp.float32, device=d).block_until_ready()
            return
        except Exception:
            if attempt == 2:
                raise


def run_on_device(inputs, rows_total=B, trace=False, repeats=1, **build_kw):
    """Shard, run on 8 cores, gather. Returns (out [rows_total], results obj)."""
    from concourse.bass_utils import run_bass_kernel_spmd

    _probe_devices()

    rows_core = rows_total // NCORES
    key = (rows_core, repeats, tuple(sorted(build_kw.items())))
    if key not in _CACHE:
        _CACHE[key] = build_nc(rows_core, repeats=repeats, **build_kw)
    nc = _CACHE[key]

    w0, w1, w2, w3, bs, avec = _prep_host(inputs)
    ob = np.ascontiguousarray(np.asarray(inputs["observations"], np.float32)[:rows_total])
    gl = np.ascontiguousarray(np.asarray(inputs["goals"], np.float32)[:rows_total])
    in_maps = []
    for c in range(NCORES):
        sl = slice(c * rows_core, (c + 1) * rows_core)
        in_maps.append({
            "observations": ob[sl], "goals": gl[sl],
            "w0": w0, "w1": w1, "w2": w2, "w3": w3, "bs": bs, "avec": avec,
        })
    r = run_bass_kernel_spmd(nc, in_maps, list(range(NCORES)), trace=trace)
    outp = np.concatenate([r.results[c]["out"] for c in range(NCORES)])
    return outp, r


def kernel(**inputs):
    out, _ = run_on_device(inputs)
    return out.astype(np.float32)



# revision 8
# speedup vs baseline: 1.2680x; 1.2680x over previous
"""Trainium2 Bass kernel for nn_GCIQEValue (MLP + IQE head), 8-core data parallel.

Math (validated vs reference):
  phi(x) = LN-MLP: 3x [matmul+bias -> tanh-gelu -> LayerNorm(affine folded into
  next W on host)] then final matmul+bias.
  IQE per row, per 32-dim component c with x = phi_s[c], y = phi_g[c]:
    y' = max(x, y)                      (interval [x_i, max(x_i,y_i)])
    u = sort(x), v = sort(y')           (independent keys-only sorts: the
                                         union-measure depends only on the
                                         multisets of starts/ends)
    comp_c = sum(v) - u_0 - sum_{i>=1} max(u_i, v_{i-1})
  out = sig(alpha) * mean_c(comp) + (1 - sig(alpha)) * max_c(comp)

Structure: 7-stage software pipeline (For_i_pipelined) over 128-row tiles:
  S0 load | S1 L0 | S2 L1 | S3 L2 | S4 L3+ymax | S5 sort p0-7 | S6 sort p8-14+post
LN statistics ride ACT accum_out (sum of gelu / sum of squares); the bitonic
sort runs on DVE as strided min/max tensor_tensor pairs.
"""

import numpy as np

B = 131072
OBS = 64
H = 512
NCOMP = 16
DPC = 32
NCORES = 8
P = 128
LN_EPS = 1e-6

_CACHE = {}

# bitonic schedule for 32-wide ascending sort: 15 passes
_SCHED = [("pair", 0, 0)]
for _L in (4, 8, 16, 32):
    _SCHED.append(("flip", _L, 0))
    _d = _L // 4
    while _d >= 1:
        _SCHED.append(("shift", _L, _d))
        _d //= 2


# ---------------------------------------------------------------- device kernel
def build_nc(rows_per_core=B // NCORES, unroll=4, gelu="hw", repeats=1,
             stage_bufs=None, mlp_bufs=3, psum_bufs=4, split_pass=7,
             n_passes=15, sort_chunks=1, n_layers=3, ln_lite=False,
             rsqrt_newton=True, bias_mode="pe", split_l0=True, hints=False,
             mm_dt="fp32r"):
    """Build the Bass (Bacc) module for one core processing rows_per_core rows."""
    import concourse.bass as bass
    import concourse.mybir as mybir
    import concourse.tile as tile
    from concourse import bacc
    from concourse.masks import make_identity

    fp32 = mybir.dt.float32
    fp32r = mybir.dt.float32r
    AT = mybir.ActivationFunctionType
    OP = mybir.AluOpType

    def _r(ap):
        """View an fp32 AP as float32r for fast single-pass PE matmul."""
        return ap.bitcast(fp32r) if mm_dt in ("fp32r", "fp32r_tr") else ap

    def _tr(ap):
        """fp32r view for transpose identities (1.5 vs 2.0 cyc/row)."""
        return ap.bitcast(fp32r) if mm_dt == "fp32r_tr" else ap

    nt = rows_per_core // P
    assert rows_per_core % P == 0
    if stage_bufs is None:
        stage_bufs = unroll

    nc = bacc.Bacc("TRN2", target_bir_lowering=False, debug=False)

    obs = nc.declare_dram_parameter("observations", [rows_per_core, OBS], fp32,
                                    isOutput=False)
    gls = nc.declare_dram_parameter("goals", [rows_per_core, OBS], fp32,
                                    isOutput=False)
    w0d = nc.declare_dram_parameter("w0", [OBS, H], fp32, isOutput=False)
    w1d = nc.declare_dram_parameter("w1", [H, H], fp32, isOutput=False)
    w2d = nc.declare_dram_parameter("w2", [H, H], fp32, isOutput=False)
    w3d = nc.declare_dram_parameter("w3", [H, H], fp32, isOutput=False)
    bsd = nc.declare_dram_parameter("bs", [4, H], fp32, isOutput=False)
    avd = nc.declare_dram_parameter("avec", [P, 2], fp32, isOutput=False)
    onesd = nc.declare_dram_parameter("onesv", [1, P], fp32, isOutput=False)
    out = nc.declare_dram_parameter("out", [rows_per_core], fp32, isOutput=True)

    obs_v = obs[:].rearrange("(n p) f -> n p f", p=P)
    gls_v = gls[:].rearrange("(n p) f -> n p f", p=P)
    out_v = out[:].rearrange("(n p) -> n p", p=P)

    gelu_f = AT.Gelu_apprx_tanh if gelu == "hw" else AT.Identity

    with tile.TileContext(nc) as tc:
        with (
            tc.tile_pool(name="const", bufs=1) as cpool,
            tc.tile_pool(name="mlp", bufs=mlp_bufs) as mp,
            tc.tile_pool(name="srt", bufs=mlp_bufs) as sp,
            tc.tile_pool(name="pipe", bufs=1) as pipe_pool,
            tc.tile_pool(name="ps", bufs=psum_bufs, space="PSUM") as pp,
            tc.tile_pool(name="pst", bufs=8 - psum_bufs, space="PSUM") as ppt,
        ):
            # ---- constants
            mmdt = fp32r if mm_dt in ("fp32r", "fp32r_tr") else fp32
            w0 = cpool.tile([OBS, H], mmdt)
            nc.sync.dma_start(out=w0, in_=w0d[:].bitcast(mmdt))
            wl = []
            for wd, nm in ((w1d, "w1"), (w2d, "w2"), (w3d, "w3")):
                t = cpool.tile([P, 4, H], mmdt, tag=nm)
                nc.sync.dma_start(
                    out=t, in_=wd[:].rearrange("(c p) n -> p c n", p=P).bitcast(mmdt))
                wl.append(t)
            bsc = cpool.tile([1, 4, H], mmdt)
            nc.sync.dma_start(
                out=bsc, in_=bsd[:].rearrange("(o c) n -> o c n", o=1).bitcast(mmdt))
            if bias_mode != "pe":
                bsb = cpool.tile([P, 4, H], fp32)
                nc.sync.dma_start(
                    out=bsb,
                    in_=bass.AP(tensor=bsd[:].tensor, offset=0,
                                ap=[[0, P]] + list(bsd[:].ap)))
            avec = cpool.tile([P, 2], fp32)
            nc.sync.dma_start(out=avec, in_=avd[:])
            ident = cpool.tile([P, P], fp32)
            make_identity(nc, ident)
            ones = cpool.tile([1, P], mmdt)
            nc.sync.dma_start(out=ones, in_=onesd[:].bitcast(mmdt))
            epst = cpool.tile([P, 1], fp32)
            nc.vector.memset(epst, LN_EPS)

            def matmul_from(t_sb, li):
                """t_sb [128, F_in] row-major -> pz PSUM [128, 512] for layer li
                (li = 0 uses w0/64-wide input, else wl[li-1])."""
                pz = pp.tile([P, H], fp32, tag="pz")
                start = True
                if bias_mode != "pe":
                    eng = nc.scalar if bias_mode == "act" else nc.vector
                    if bias_mode == "act":
                        eng.copy(pz, bsb[:, li, :])
                    else:
                        eng.tensor_copy(pz, bsb[:, li, :])
                    start = False
                if li == 0:
                    pTf = ppt.tile([P, H], fp32, tag="pT")
                    nc.tensor.transpose(pTf[0:OBS, 0:P], t_sb, _tr(ident))
                    xT = mp.tile([OBS, P], mmdt, tag="xT")
                    nc.scalar.copy(xT, pTf[0:OBS, 0:P])
                    nc.tensor.matmul(pz, xT, w0, start=start,
                                     stop=(bias_mode != "pe"))
                else:
                    pTf = ppt.tile([P, H], fp32, tag="pT")
                    for k in range(4):
                        nc.tensor.transpose(pTf[:, k * P:(k + 1) * P],
                                            t_sb[:, k * P:(k + 1) * P],
                                            _tr(ident))
                    tT = mp.tile([P, 4, P], mmdt, tag="tT")
                    nc.scalar.copy(tT, pTf)
                    for k in range(4):
                        nc.tensor.matmul(pz, tT[:, k, :],
                                         wl[li - 1][:, k, :],
                                         start=(start and k == 0),
                                         stop=(bias_mode != "pe" and k == 3))
                if bias_mode == "pe":
                    nc.tensor.matmul(pz, ones, bsc[:, li, :],
                                     start=False, stop=True)
                return pz

            def gelu_ln(pz, t_out):
                """pz PSUM -> t_out SBUF: LayerNorm(gelu(pz)) via ACT stats."""
                if ln_lite:
                    nc.scalar.activation(t_out, pz, gelu_f)
                    return
                g = mp.tile([P, H], fp32, tag="g")
                sums = mp.tile([P, 2], fp32, tag="sums")
                nc.scalar.activation(g, pz, gelu_f, accum_out=sums[:, 0:1])
                gsq = mp.tile([P, H], fp32, tag="gsq")
                nc.scalar.activation(gsq, g, AT.Square, accum_out=sums[:, 1:2])
                mv2 = mp.tile([P, 2], fp32, tag="mv2")
                nc.vector.tensor_scalar_mul(mv2, sums, 1.0 / H)
                msq = mp.tile([P, 1], fp32, tag="msq")
                nc.vector.tensor_tensor(out=msq, in0=mv2[:, 0:1],
                                        in1=mv2[:, 0:1], op=OP.mult)
                varb = mp.tile([P, 1], fp32, tag="varb")
                nc.vector.tensor_tensor(out=varb, in0=mv2[:, 1:2], in1=msq,
                                        op=OP.subtract)
                nc.vector.tensor_scalar_add(varb, varb, LN_EPS)
                rstd = mp.tile([P, 1], fp32, tag="rstd")
                if rsqrt_newton:
                    # rsqrt without the ACT Sqrt table set: quake seed on DVE
                    # int ALU + 3 Newton iterations (rel err ~1e-7).
                    i32 = mybir.dt.int32
                    yi = mp.tile([P, 1], i32, tag="yi")
                    nc.vector.tensor_scalar(
                        out=yi, in0=varb.bitcast(i32), scalar1=1,
                        scalar2=None, op0=OP.logical_shift_right)
                    nc.vector.tensor_scalar(
                        out=yi, in0=yi, scalar1=-1, scalar2=0x5F3759DF,
                        op0=OP.mult, op1=OP.add)
                    y = yi.bitcast(fp32)
                    t1 = mp.tile([P, 1], fp32, tag="nt1")
                    for _ in range(3):
                        nc.vector.tensor_tensor(out=t1, in0=varb, in1=y,
                                                op=OP.mult)
                        nc.vector.tensor_tensor(out=t1, in0=t1, in1=y,
                                                op=OP.mult)
                        nc.vector.tensor_scalar(out=t1, in0=t1, scalar1=-0.5,
                                                scalar2=1.5, op0=OP.mult,
                                                op1=OP.add)
                        nc.vector.tensor_tensor(out=y, in0=y, in1=t1,
                                                op=OP.mult)
                    rstd = y
                else:
                    std = mp.tile([P, 1], fp32, tag="std")
                    nc.scalar.activation(std, varb, AT.Sqrt)
                    nc.vector.reciprocal(rstd, std)
                nmr = mp.tile([P, 1], fp32, tag="nmr")
                nc.vector.scalar_tensor_tensor(out=nmr, in0=mv2[:, 0:1],
                                               scalar=-1.0, in1=rstd,
                                               op0=OP.mult, op1=OP.mult)
                nc.scalar.activation(t_out, g, AT.Identity, bias=nmr, scale=rstd)

            def emit_sort_pass(p_idx, src_x, src_y, dst):
                """Emit bitonic pass p_idx. Pass 0 reads (src_x, src_y) pair
                tensors; later passes read src_x as the full [P,1024] buffer.
                sort_chunks splits each instruction along the group dim to
                amortize the DVE post-op DRAIN (cost ~ 2*dur - const)."""
                kind, L, d = _SCHED[p_idx]
                V = nc.vector
                C = sort_chunks

                def ch(view, c):
                    n = view.shape[1]
                    return view[:, c * n // C:(c + 1) * n // C]

                if kind == "pair":
                    for src, off in ((src_x, 0), (src_y, H)):
                        s = src.rearrange("p (g e) -> p g e", e=DPC)
                        o = dst[:, off:off + H].rearrange("p (g e) -> p g e",
                                                          e=DPC)
                        for c in range(C):
                            sc, oc = ch(s, c), ch(o, c)
                            V.tensor_tensor(out=oc[:, :, 0::2],
                                            in0=sc[:, :, 0::2],
                                            in1=sc[:, :, 1::2], op=OP.min)
                            V.tensor_tensor(out=oc[:, :, 1::2],
                                            in0=sc[:, :, 0::2],
                                            in1=sc[:, :, 1::2], op=OP.max)
                elif kind == "flip":
                    half = L // 2
                    s = src_x.rearrange("p (b e) -> p b e", e=L)
                    o = dst.rearrange("p (b e) -> p b e", e=L)
                    for c in range(C):
                        sc, oc = ch(s, c), ch(o, c)
                        V.tensor_tensor(out=oc[:, :, 0:half],
                                        in0=sc[:, :, 0:half],
                                        in1=sc[:, :, L - 1:half - 1:-1],
                                        op=OP.min)
                        V.tensor_tensor(out=oc[:, :, half:L],
                                        in0=sc[:, :, half:L],
                                        in1=sc[:, :, half - 1::-1], op=OP.max)
                else:
                    s = src_x.rearrange("p (c e) -> p c e", e=2 * d)
                    o = dst.rearrange("p (c e) -> p c e", e=2 * d)
                    for c in range(C):
                        sc, oc = ch(s, c), ch(o, c)
                        V.tensor_tensor(out=oc[:, :, 0:d], in0=sc[:, :, 0:d],
                                        in1=sc[:, :, d:2 * d], op=OP.min)
                        V.tensor_tensor(out=oc[:, :, d:2 * d],
                                        in0=sc[:, :, 0:d],
                                        in1=sc[:, :, d:2 * d], op=OP.max)

            # ---------------- pipeline stages
            def st_load(pipe, iv):
                xt = pipe.intermediate_tile([P, OBS], fp32, name="xt")
                gt = pipe.intermediate_tile([P, OBS], fp32, name="gt")
                nc.sync.dma_start(out=xt, in_=obs_v[iv])
                nc.sync.dma_start(out=gt, in_=gls_v[iv])
                return (xt, gt)

            def mk_layer(li):
                def st(pipe, iv, prev):
                    to, tg = prev
                    if n_layers < 3:  # ablation: copy-through this mid layer
                        oo = pipe.intermediate_tile([P, H], fp32, name=f"to{li}")
                        og = pipe.intermediate_tile([P, H], fp32, name=f"tg{li}")
                        nc.scalar.copy(oo, to)
                        nc.scalar.copy(og, tg)
                        return (oo, og)
                    oo = pipe.intermediate_tile([P, H], fp32, name=f"to{li}")
                    og = pipe.intermediate_tile([P, H], fp32, name=f"tg{li}")
                    gelu_ln(matmul_from(to, li), oo)
                    gelu_ln(matmul_from(tg, li), og)
                    return (oo, og)
                return st

            def st_l3(pipe, iv, prev):
                to, tg = prev
                phis = pipe.intermediate_tile([P, H], fp32, name="phis")
                pz = matmul_from(to, 3)
                nc.scalar.copy(phis, pz)
                pzg = matmul_from(tg, 3)
                ypr = pipe.intermediate_tile([P, H], fp32, name="ypr")
                nc.vector.tensor_tensor(out=ypr, in0=phis, in1=pzg, op=OP.max)
                return (phis, ypr)

            def st_sort_a(pipe, iv, prev):
                phis, ypr = prev
                bufA = pipe.intermediate_tile([P, 2 * H], fp32, name="bufA")
                bufB = pipe.intermediate_tile([P, 2 * H], fp32, name="bufB")
                emit_sort_pass(0, phis, ypr, bufA)
                cur, nxt = bufA, bufB
                for pidx in range(1, split_pass):
                    if pidx < n_passes:
                        emit_sort_pass(pidx, cur, None, nxt)
                    cur, nxt = nxt, cur
                return (bufA, bufB)

            def st_sort_b(pipe, iv, prev):
                bufA, bufB = prev
                cur, nxt = (bufB, bufA) if split_pass % 2 == 0 else (bufA, bufB)
                for pidx in range(split_pass, 15):
                    if pidx < n_passes:
                        emit_sort_pass(pidx, cur, None, nxt)
                    cur, nxt = nxt, cur
                fin = cur  # pass 14 (even) -> bufA when split parity works out
                fv = fin.rearrange("p (h g e) -> p h g e", h=2, e=DPC)
                # coupling: u[i] <- max(u[i], v[i-1]) for i>=1, in place
                nc.vector.tensor_tensor(out=fv[:, 0, :, 1:DPC],
                                        in0=fv[:, 0, :, 1:DPC],
                                        in1=fv[:, 1, :, 0:DPC - 1], op=OP.max)
                red = sp.tile([P, 2, NCOMP], fp32, tag="red")
                nc.vector.tensor_reduce(out=red, in_=fv,
                                        axis=mybir.AxisListType.X, op=OP.add)
                comp = sp.tile([P, NCOMP], fp32, tag="comp")
                nc.vector.tensor_tensor(out=comp, in0=red[:, 1, :],
                                        in1=red[:, 0, :], op=OP.subtract)
                cs = sp.tile([P, 1], fp32, tag="cs")
                nc.vector.tensor_reduce(out=cs, in_=comp,
                                        axis=mybir.AxisListType.X, op=OP.add)
                cm = sp.tile([P, 1], fp32, tag="cm")
                nc.vector.tensor_reduce(out=cm, in_=comp,
                                        axis=mybir.AxisListType.X, op=OP.max)
                res = sp.tile([P, 1], fp32, tag="res")
                nc.vector.tensor_scalar(out=res, in0=cs, scalar1=avec[:, 0:1],
                                        scalar2=None, op0=OP.mult)
                nc.vector.scalar_tensor_tensor(out=res, in0=cm,
                                               scalar=avec[:, 1:2], in1=res,
                                               op0=OP.mult, op1=OP.add)
                nc.sync.dma_start(out=out_v[iv], in_=res[:, 0:1])

            def st_l01(pipe, iv, prev):
                xt, gt = prev
                t0o = mp.tile([P, H], fp32, tag="t0o")
                t0g = mp.tile([P, H], fp32, tag="t0g")
                gelu_ln(matmul_from(xt, 0), t0o)
                gelu_ln(matmul_from(gt, 0), t0g)
                oo = pipe.intermediate_tile([P, H], fp32, name="to1")
                og = pipe.intermediate_tile([P, H], fp32, name="tg1")
                gelu_ln(matmul_from(t0o, 1), oo)
                gelu_ln(matmul_from(t0g, 1), og)
                return (oo, og)

            def st_l0(pipe, iv, prev):
                xt, gt = prev
                oo = pipe.intermediate_tile([P, H], fp32, name="to0")
                og = pipe.intermediate_tile([P, H], fp32, name="tg0")
                gelu_ln(matmul_from(xt, 0), oo)
                gelu_ln(matmul_from(gt, 0), og)
                return (oo, og)

            if split_l0:
                stages = [st_load, st_l0, mk_layer(1), mk_layer(2), st_l3,
                          st_sort_a, st_sort_b]
            else:
                stages = [st_load, st_l01, mk_layer(2), st_l3,
                          st_sort_a, st_sort_b]

            def run_pipe():
                he = (mybir.EngineType.PE, mybir.EngineType.DVE,
                      mybir.EngineType.Activation, mybir.EngineType.SP,
                      mybir.EngineType.Pool) if hints else ()
                tc.For_i_pipelined(stages, 0, nt, 1, pool=pipe_pool,
                                   unroll=unroll, staged_num_bufs=stage_bufs,
                                   hint_engines=he)

            if repeats == 1:
                run_pipe()
            else:
                with tc.For_i(0, repeats, 1):
                    run_pipe()

    nc.finalize()
    return nc


# ---------------------------------------------------------------- host wrapper
def _prep_host(inputs):
    """Fold LN affine params into the following layer's weights; build avec."""
    f32 = np.float32
    W0 = np.asarray(inputs["W0"], f32)
    b0 = np.asarray(inputs["b0"], f32)
    w, b = [W0], [b0]
    for i in (0, 1, 2):
        s = np.asarray(inputs[f"ln{i}_s"], f32)
        t = np.asarray(inputs[f"ln{i}_b"], f32)
        Wn = np.asarray(inputs[("W1", "W2", "W3")[i]], f32)
        bn = np.asarray(inputs[("b1", "b2", "b3")[i]], f32)
        w.append(s[:, None] * Wn)
        b.append(bn + t @ Wn)
    bs = np.stack(b, 0)  # [4, 512]
    alpha = float(np.asarray(inputs["alpha"]))
    a = 1.0 / (1.0 + np.exp(-alpha))
    avec = np.empty((P, 2), f32)
    avec[:, 0] = a / NCOMP
    avec[:, 1] = 1.0 - a
    return w[0], w[1], w[2], w[3], bs.astype(f32), avec


def _probe_devices():
    """Poke every core with a tiny op; retries to shake off a stale
    NRT_EXEC_UNIT_UNRECOVERABLE state left by a previous process."""
    import jax
    import jax.numpy as jnp

    for attempt in range(3):
        try:
            for d in jax.devices()[:NCORES]:
                jnp.zeros((1,), jnp.float32, device=d).block_until_ready()
            return
        except Exception:
            if attempt == 2:
                raise


def run_on_device(inputs, rows_total=B, trace=False, repeats=1, **build_kw):
    """Shard, run on 8 cores, gather. Returns (out [rows_total], results obj)."""
    from concourse.bass_utils import run_bass_kernel_spmd

    _probe_devices()

    rows_core = rows_total // NCORES
    key = (rows_core, repeats, tuple(sorted(build_kw.items())))
    if key not in _CACHE:
        _CACHE[key] = build_nc(rows_core, repeats=repeats, **build_kw)
    nc = _CACHE[key]

    w0, w1, w2, w3, bs, avec = _prep_host(inputs)
    ob = np.ascontiguousarray(np.asarray(inputs["observations"], np.float32)[:rows_total])
    gl = np.ascontiguousarray(np.asarray(inputs["goals"], np.float32)[:rows_total])
    in_maps = []
    for c in range(NCORES):
        sl = slice(c * rows_core, (c + 1) * rows_core)
        in_maps.append({
            "observations": ob[sl], "goals": gl[sl],
            "w0": w0, "w1": w1, "w2": w2, "w3": w3, "bs": bs, "avec": avec,
            "onesv": np.ones((1, P), np.float32),
        })
    r = run_bass_kernel_spmd(nc, in_maps, list(range(NCORES)), trace=trace)
    outp = np.concatenate([r.results[c]["out"] for c in range(NCORES)])
    return outp, r


def kernel(**inputs):
    out, _ = run_on_device(inputs)
    return out.astype(np.float32)



# revision 18
# speedup vs baseline: 1.9369x; 1.5275x over previous
"""Trainium2 Bass kernel for nn_GCIQEValue (MLP + IQE head), 8-core data parallel.

Math (validated vs reference):
  phi(x) = LN-MLP: 3x [matmul+bias -> tanh-gelu -> LayerNorm(affine folded into
  next W on host)] then final matmul+bias.
  IQE per row, per 32-dim component c with x = phi_s[c], y = phi_g[c]:
    y' = max(x, y)                      (interval [x_i, max(x_i,y_i)])
    u = sort(x), v = sort(y')           (independent keys-only sorts: the
                                         union-measure depends only on the
                                         multisets of starts/ends)
    comp_c = sum(v) - u_0 - sum_{i>=1} max(u_i, v_{i-1})
  out = sig(alpha) * mean_c(comp) + (1 - sig(alpha)) * max_c(comp)

Structure: 7-stage software pipeline (For_i_pipelined) over 128-row tiles:
  S0 load | S1 L0 | S2 L1 | S3 L2 | S4 L3+ymax | S5 sort p0-7 | S6 sort p8-14+post
LN statistics ride ACT accum_out (sum of gelu / sum of squares); the bitonic
sort runs on DVE as strided min/max tensor_tensor pairs.
"""

import numpy as np

B = 131072
OBS = 64
H = 512
NCOMP = 16
DPC = 32
NCORES = 8
P = 128
LN_EPS = 1e-6

_CACHE = {}

# bitonic schedule for 32-wide ascending sort: 15 passes
_SCHED = [("pair", 0, 0)]
for _L in (4, 8, 16, 32):
    _SCHED.append(("flip", _L, 0))
    _d = _L // 4
    while _d >= 1:
        _SCHED.append(("shift", _L, _d))
        _d //= 2


# ---------------------------------------------------------------- device kernel
def build_nc(rows_per_core=B // NCORES, unroll=4, gelu="hw", repeats=1,
             stage_bufs=None, mlp_bufs=3, psum_bufs=4, split_pass=7,
             n_passes=15, sort_chunks=1, n_layers=3, ln_lite=False,
             rsqrt_newton=True, bias_mode="pe", split_l0=True, hints=False,
             mm_dt="fp32r", newton=3, pool_v=0, pool_neg=True, sort_dt="fp32",
             act_dt="fp32"):
    """Build the Bass (Bacc) module for one core processing rows_per_core rows."""
    import concourse.bass as bass
    import concourse.mybir as mybir
    import concourse.tile as tile
    from concourse import bacc
    from concourse.masks import make_identity

    fp32 = mybir.dt.float32
    fp32r = mybir.dt.float32r
    AT = mybir.ActivationFunctionType
    OP = mybir.AluOpType

    def _r(ap):
        """View an fp32 AP as float32r for fast single-pass PE matmul."""
        return ap.bitcast(fp32r) if mm_dt in ("fp32r", "fp32r_tr") else ap

    def _tr(ap):
        """fp32r view for transpose identities (1.5 vs 2.0 cyc/row)."""
        return ap.bitcast(fp32r) if mm_dt == "fp32r_tr" else ap

    nt = rows_per_core // P
    assert rows_per_core % P == 0
    if stage_bufs is None:
        stage_bufs = unroll

    nc = bacc.Bacc("TRN2", target_bir_lowering=False, debug=False)

    obs = nc.declare_dram_parameter("observations", [rows_per_core, OBS], fp32,
                                    isOutput=False)
    gls = nc.declare_dram_parameter("goals", [rows_per_core, OBS], fp32,
                                    isOutput=False)
    wdt = mybir.dt.bfloat16 if act_dt == "bf16" else fp32
    w0d = nc.declare_dram_parameter("w0", [OBS, H], wdt, isOutput=False)
    w1d = nc.declare_dram_parameter("w1", [H, H], wdt, isOutput=False)
    w2d = nc.declare_dram_parameter("w2", [H, H], wdt, isOutput=False)
    w3d = nc.declare_dram_parameter("w3", [H, H], wdt, isOutput=False)
    bsd = nc.declare_dram_parameter("bs", [4, H], wdt, isOutput=False)
    avd = nc.declare_dram_parameter("avec", [P, 2], fp32, isOutput=False)
    onesd = nc.declare_dram_parameter("onesv", [1, P], wdt, isOutput=False)
    wsd = nc.declare_dram_parameter("ws", [3, 2, H], wdt, isOutput=False)
    out = nc.declare_dram_parameter("out", [rows_per_core], fp32, isOutput=True)

    obs_v = obs[:].rearrange("(n p) f -> n p f", p=P)
    gls_v = gls[:].rearrange("(n p) f -> n p f", p=P)
    out_v = out[:].rearrange("(n p) -> n p", p=P)

    gelu_f = AT.Gelu_apprx_tanh if gelu == "hw" else AT.Identity
    bf16 = mybir.dt.bfloat16

    with tile.TileContext(nc) as tc:
        with (
            tc.tile_pool(name="const", bufs=1) as cpool,
            tc.tile_pool(name="mlp", bufs=mlp_bufs) as mp,
            tc.tile_pool(name="srt", bufs=mlp_bufs) as sp,
            tc.tile_pool(name="pipe", bufs=1) as pipe_pool,
            tc.tile_pool(name="ps", bufs=psum_bufs, space="PSUM") as pp,
            tc.tile_pool(name="pst", bufs=8 - psum_bufs, space="PSUM") as ppt,
        ):
            # ---- constants
            if act_dt == "bf16":
                mmdt = bf16
                adt = bf16
            else:
                mmdt = fp32r if mm_dt in ("fp32r", "fp32r_tr") else fp32
                adt = fp32
            sort_t = bf16 if sort_dt == "bf16" else fp32
            _cast = (lambda ap: ap) if act_dt == "bf16" else \
                (lambda ap: ap.bitcast(mmdt))
            w0 = cpool.tile([OBS, H], mmdt)
            nc.sync.dma_start(out=w0, in_=_cast(w0d[:]))
            wl = []
            for wd, nm in ((w1d, "w1"), (w2d, "w2"), (w3d, "w3")):
                t = cpool.tile([P, 4, H], mmdt, tag=nm)
                nc.sync.dma_start(
                    out=t, in_=_cast(wd[:].rearrange("(c p) n -> p c n", p=P)))
                wl.append(t)
            bsc = cpool.tile([1, 4, H], mmdt)
            nc.sync.dma_start(
                out=bsc, in_=_cast(bsd[:].rearrange("(o c) n -> o c n", o=1)))
            if bias_mode != "pe":
                bsb = cpool.tile([P, 4, H], fp32)
                nc.sync.dma_start(
                    out=bsb,
                    in_=bass.AP(tensor=bsd[:].tensor, offset=0,
                                ap=[[0, P]] + list(bsd[:].ap)))
            avec = cpool.tile([P, 2], fp32)
            nc.sync.dma_start(out=avec, in_=avd[:])
            wsc = cpool.tile([2, 3, H], mmdt, tag="wsc")
            nc.sync.dma_start(
                out=wsc, in_=_cast(wsd[:].rearrange("l t n -> t l n")))
            ident = cpool.tile([P, P], fp32)
            make_identity(nc, ident)
            if act_dt == "bf16":
                identb = cpool.tile([P, P], bf16)
                make_identity(nc, identb)
            else:
                identb = ident
            ones = cpool.tile([1, P], mmdt)
            nc.sync.dma_start(out=ones, in_=_cast(onesd[:]))
            epst = cpool.tile([P, 1], fp32)
            nc.vector.memset(epst, LN_EPS)

            def matmul_from(t_sb, li, aux_lhsT=None):
                """t_sb [128, F_in] row-major -> pz PSUM [128, 512] for layer
                li (li = 0 uses w0/64-wide input, else wl[li-1]). With
                aux_lhsT [2, P] = [-mean; 1/rstd] of the previous LN, adds the
                rank-2 term aux_lhsT.T @ [wsum_li; b_li] so the caller can fold
                the LN affine into the next gelu's per-row scale."""
                pz = pp.tile([P, H], fp32, tag="pz")
                start = True
                if bias_mode != "pe":
                    eng = nc.scalar if bias_mode == "act" else nc.vector
                    if bias_mode == "act":
                        eng.copy(pz, bsb[:, li, :])
                    else:
                        eng.tensor_copy(pz, bsb[:, li, :])
                    start = False
                if li == 0:
                    pTf = ppt.tile([P, H], fp32, tag="pT")
                    nc.tensor.transpose(pTf[0:OBS, 0:P], t_sb, _tr(ident))
                    xT = mp.tile([OBS, P], mmdt, tag="xT")
                    nc.scalar.copy(xT, pTf[0:OBS, 0:P])
                    nc.tensor.matmul(pz, xT, w0, start=start,
                                     stop=(bias_mode != "pe"))
                else:
                    pTf = ppt.tile([P, H], adt, tag="pT")
                    for k in range(4):
                        nc.tensor.transpose(pTf[:, k * P:(k + 1) * P],
                                            t_sb[:, k * P:(k + 1) * P],
                                            _tr(identb) if act_dt != "bf16"
                                            else identb)
                    tT = mp.tile([P, 4, P], mmdt, tag="tT")
                    nc.scalar.copy(tT, pTf)
                    for k in range(4):
                        nc.tensor.matmul(pz, tT[:, k, :],
                                         wl[li - 1][:, k, :],
                                         start=(start and k == 0),
                                         stop=(bias_mode != "pe" and k == 3))
                if aux_lhsT is not None:
                    nc.tensor.matmul(pz, aux_lhsT, wsc[:, li - 1, :],
                                     start=False, stop=True)
                elif bias_mode == "pe":
                    nc.tensor.matmul(pz, ones, bsc[:, li, :],
                                     start=False, stop=True)
                return pz

            def gelu_ln(pz, t_out):
                """pz PSUM -> t_out SBUF: LayerNorm(gelu(pz)) via ACT stats."""
                if ln_lite:
                    nc.scalar.activation(t_out, pz, gelu_f)
                    return
                g = mp.tile([P, H], fp32, tag="g")
                sums = mp.tile([P, 2], fp32, tag="sums")
                nc.scalar.activation(g, pz, gelu_f, accum_out=sums[:, 0:1])
                gsq = mp.tile([P, H], fp32, tag="gsq")
                nc.scalar.activation(gsq, g, AT.Square, accum_out=sums[:, 1:2])
                mv2 = mp.tile([P, 2], fp32, tag="mv2")
                nc.vector.tensor_scalar_mul(mv2, sums, 1.0 / H)
                msq = mp.tile([P, 1], fp32, tag="msq")
                nc.vector.tensor_tensor(out=msq, in0=mv2[:, 0:1],
                                        in1=mv2[:, 0:1], op=OP.mult)
                varb = mp.tile([P, 1], fp32, tag="varb")
                nc.vector.tensor_tensor(out=varb, in0=mv2[:, 1:2], in1=msq,
                                        op=OP.subtract)
                nc.vector.tensor_scalar_add(varb, varb, LN_EPS)
                rstd = mp.tile([P, 1], fp32, tag="rstd")
                if rsqrt_newton:
                    # rsqrt without the ACT Sqrt table set: quake seed on DVE
                    # int ALU + 3 Newton iterations (rel err ~1e-7).
                    i32 = mybir.dt.int32
                    yi = mp.tile([P, 1], i32, tag="yi")
                    nc.vector.tensor_scalar(
                        out=yi, in0=varb.bitcast(i32), scalar1=1,
                        scalar2=None, op0=OP.logical_shift_right)
                    nc.vector.tensor_scalar(
                        out=yi, in0=yi, scalar1=-1, scalar2=0x5F3759DF,
                        op0=OP.mult, op1=OP.add)
                    y = yi.bitcast(fp32)
                    t1 = mp.tile([P, 1], fp32, tag="nt1")
                    for _ in range(3):
                        nc.vector.tensor_tensor(out=t1, in0=varb, in1=y,
                                                op=OP.mult)
                        nc.vector.tensor_tensor(out=t1, in0=t1, in1=y,
                                                op=OP.mult)
                        nc.vector.tensor_scalar(out=t1, in0=t1, scalar1=-0.5,
                                                scalar2=1.5, op0=OP.mult,
                                                op1=OP.add)
                        nc.vector.tensor_tensor(out=y, in0=y, in1=t1,
                                                op=OP.mult)
                    rstd = y
                else:
                    std = mp.tile([P, 1], fp32, tag="std")
                    nc.scalar.activation(std, varb, AT.Sqrt)
                    nc.vector.reciprocal(rstd, std)
                nmr = mp.tile([P, 1], fp32, tag="nmr")
                nc.vector.scalar_tensor_tensor(out=nmr, in0=mv2[:, 0:1],
                                               scalar=-1.0, in1=rstd,
                                               op0=OP.mult, op1=OP.mult)
                nc.scalar.activation(t_out, g, AT.Identity, bias=nmr, scale=rstd)

            def gelu_stats(pipe, li, pz_o, pz_g, scale=None):
                """gelu (+ folded prev-LN per-row scale) and LN statistics.
                Returns (g_o, g_g, rstd [P,2], nmstT [4,P]) with nmstT rows
                [-mean_o, 1/rstd_o, -mean_g, 1/rstd_g] for the next layer's
                rank-2 bias/shift matmul."""
                i32 = mybir.dt.int32
                g_o = pipe.intermediate_tile([P, H], adt, name=f"g{li}o")
                g_g = pipe.intermediate_tile([P, H], adt, name=f"g{li}g")
                sums = mp.tile([P, 4], fp32, tag="sums4")
                sc_o = scale[:, 0:1] if scale is not None else None
                sc_g = scale[:, 1:2] if scale is not None else None
                nc.scalar.activation(g_o, pz_o, gelu_f, scale=sc_o,
                                     accum_out=sums[:, 0:1])
                nc.scalar.activation(g_g, pz_g, gelu_f, scale=sc_g,
                                     accum_out=sums[:, 1:2])
                gsq = mp.tile([P, H], adt, tag="gsq_o")
                nc.scalar.activation(gsq, g_o, AT.Square, accum_out=sums[:, 2:3])
                gsq2 = mp.tile([P, H], adt, tag="gsq_g")
                nc.scalar.activation(gsq2, g_g, AT.Square, accum_out=sums[:, 3:4])
                mean = mp.tile([P, 2], fp32, tag="mean2")
                nc.vector.tensor_scalar_mul(mean, sums[:, 0:2], 1.0 / H)
                msq = mp.tile([P, 2], fp32, tag="msq2")
                nc.vector.tensor_tensor(out=msq, in0=mean, in1=mean, op=OP.mult)
                varb = mp.tile([P, 2], fp32, tag="varb2")
                nc.vector.scalar_tensor_tensor(out=varb, in0=sums[:, 2:4],
                                               scalar=1.0 / H, in1=msq,
                                               op0=OP.mult, op1=OP.subtract)
                nc.vector.tensor_scalar_add(varb, varb, LN_EPS)
                yi = pipe.intermediate_tile([P, 2], i32, name=f"yi{li}")
                nc.vector.tensor_scalar(
                    out=yi, in0=varb.bitcast(i32), scalar1=1,
                    scalar2=None, op0=OP.logical_shift_right)
                nc.vector.tensor_scalar(
                    out=yi, in0=yi, scalar1=-1, scalar2=0x5F3759DF,
                    op0=OP.mult, op1=OP.add)
                y = yi.bitcast(fp32)
                t1 = mp.tile([P, 2], fp32, tag="nt2")
                for _ in range(newton):
                    nc.vector.tensor_tensor(out=t1, in0=varb, in1=y, op=OP.mult)
                    nc.vector.tensor_tensor(out=t1, in0=t1, in1=y, op=OP.mult)
                    nc.vector.tensor_scalar(out=t1, in0=t1, scalar1=-0.5,
                                            scalar2=1.5, op0=OP.mult,
                                            op1=OP.add)
                    nc.vector.tensor_tensor(out=y, in0=y, in1=t1, op=OP.mult)
                rstd = y
                nmst = mp.tile([P, 4], fp32, tag="nmst")
                nc.vector.tensor_scalar_mul(nmst[:, 0:3:2], mean, -1.0)
                nc.vector.tensor_tensor(out=nmst[:, 1:4:2], in0=varb, in1=rstd,
                                        op=OP.mult)
                pT4 = ppt.tile([P, H], fp32, tag="pT4")
                nc.tensor.transpose(pT4[0:4, 0:P], nmst, ident)
                nmstT = pipe.intermediate_tile([4, P], mmdt, name=f"nT{li}")
                nc.scalar.copy(nmstT, pT4[0:4, 0:P])
                return (g_o, g_g, rstd, nmstT)

            def v_engine(p_idx):
                """Engine for v-sort pass p_idx: Pool head / DVE tail. Flip
                passes need a negative-stride in1; pool_neg=False keeps those
                on DVE."""
                kind, L, d = _SCHED[p_idx]
                if not pool_neg and kind == "flip":
                    return nc.vector
                return nc.gpsimd if p_idx < pool_v else nc.vector

            def emit_sort_half(eng, p_idx, src, dst, pair_src=None):
                """One bitonic pass over a [P, 512] buffer (16 groups of
                DPC=32) on the given engine."""
                kind, L, d = _SCHED[p_idx]
                V = eng
                if kind == "pair":
                    sv = pair_src.rearrange("p (g e) -> p g e", e=DPC)
                    ov = dst.rearrange("p (g e) -> p g e", e=DPC)
                    V.tensor_tensor(out=ov[:, :, 0::2], in0=sv[:, :, 0::2],
                                    in1=sv[:, :, 1::2], op=OP.min)
                    V.tensor_tensor(out=ov[:, :, 1::2], in0=sv[:, :, 0::2],
                                    in1=sv[:, :, 1::2], op=OP.max)
                elif kind == "flip":
                    half = L // 2
                    sv = src.rearrange("p (b e) -> p b e", e=L)
                    ov = dst.rearrange("p (b e) -> p b e", e=L)
                    V.tensor_tensor(out=ov[:, :, 0:half], in0=sv[:, :, 0:half],
                                    in1=sv[:, :, L - 1:half - 1:-1], op=OP.min)
                    V.tensor_tensor(out=ov[:, :, half:L], in0=sv[:, :, half:L],
                                    in1=sv[:, :, half - 1::-1], op=OP.max)
                else:
                    sv = src.rearrange("p (c e) -> p c e", e=2 * d)
                    ov = dst.rearrange("p (c e) -> p c e", e=2 * d)
                    V.tensor_tensor(out=ov[:, :, 0:d], in0=sv[:, :, 0:d],
                                    in1=sv[:, :, d:2 * d], op=OP.min)
                    V.tensor_tensor(out=ov[:, :, d:2 * d], in0=sv[:, :, 0:d],
                                    in1=sv[:, :, d:2 * d], op=OP.max)

            def emit_sort_pass(p_idx, src_x, src_y, dst):
                """Emit bitonic pass p_idx. Pass 0 reads (src_x, src_y) pair
                tensors; later passes read src_x as the full [P,1024] buffer.
                sort_chunks splits each instruction along the group dim to
                amortize the DVE post-op DRAIN (cost ~ 2*dur - const)."""
                kind, L, d = _SCHED[p_idx]
                V = nc.vector
                C = sort_chunks

                def ch(view, c):
                    n = view.shape[1]
                    return view[:, c * n // C:(c + 1) * n // C]

                if kind == "pair":
                    for src, off in ((src_x, 0), (src_y, H)):
                        s = src.rearrange("p (g e) -> p g e", e=DPC)
                        o = dst[:, off:off + H].rearrange("p (g e) -> p g e",
                                                          e=DPC)
                        for c in range(C):
                            sc, oc = ch(s, c), ch(o, c)
                            V.tensor_tensor(out=oc[:, :, 0::2],
                                            in0=sc[:, :, 0::2],
                                            in1=sc[:, :, 1::2], op=OP.min)
                            V.tensor_tensor(out=oc[:, :, 1::2],
                                            in0=sc[:, :, 0::2],
                                            in1=sc[:, :, 1::2], op=OP.max)
                elif kind == "flip":
                    half = L // 2
                    s = src_x.rearrange("p (b e) -> p b e", e=L)
                    o = dst.rearrange("p (b e) -> p b e", e=L)
                    for c in range(C):
                        sc, oc = ch(s, c), ch(o, c)
                        V.tensor_tensor(out=oc[:, :, 0:half],
                                        in0=sc[:, :, 0:half],
                                        in1=sc[:, :, L - 1:half - 1:-1],
                                        op=OP.min)
                        V.tensor_tensor(out=oc[:, :, half:L],
                                        in0=sc[:, :, half:L],
                                        in1=sc[:, :, half - 1::-1], op=OP.max)
                else:
                    s = src_x.rearrange("p (c e) -> p c e", e=2 * d)
                    o = dst.rearrange("p (c e) -> p c e", e=2 * d)
                    for c in range(C):
                        sc, oc = ch(s, c), ch(o, c)
                        V.tensor_tensor(out=oc[:, :, 0:d], in0=sc[:, :, 0:d],
                                        in1=sc[:, :, d:2 * d], op=OP.min)
                        V.tensor_tensor(out=oc[:, :, d:2 * d],
                                        in0=sc[:, :, 0:d],
                                        in1=sc[:, :, d:2 * d], op=OP.max)

            # ---------------- pipeline stages
            def st_load(pipe, iv):
                xt = pipe.intermediate_tile([P, OBS], fp32, name="xt")
                gt = pipe.intermediate_tile([P, OBS], fp32, name="gt")
                nc.sync.dma_start(out=xt, in_=obs_v[iv])
                nc.sync.dma_start(out=gt, in_=gls_v[iv])
                return (xt, gt)

            def mk_layer(li):
                def st(pipe, iv, prev):
                    g_o, g_g, rstd, nmstT = prev
                    pz_o = matmul_from(g_o, li, aux_lhsT=nmstT[0:2, :])
                    pz_g = matmul_from(g_g, li, aux_lhsT=nmstT[2:4, :])
                    return gelu_stats(pipe, li, pz_o, pz_g, scale=rstd)
                return st

            def st_l0(pipe, iv, prev):
                xt, gt = prev
                return gelu_stats(pipe, 0, matmul_from(xt, 0),
                                  matmul_from(gt, 0))

            def st_l3(pipe, iv, prev):
                g_o, g_g, rstd, nmstT = prev
                phis = pipe.intermediate_tile([P, H], sort_t, name="phis")
                pz = matmul_from(g_o, 3, aux_lhsT=nmstT[0:2, :])
                nc.scalar.activation(phis, pz, AT.Identity,
                                     scale=rstd[:, 0:1])
                pzg = matmul_from(g_g, 3, aux_lhsT=nmstT[2:4, :])
                ypr = pipe.intermediate_tile([P, H], sort_t, name="ypr")
                nc.vector.scalar_tensor_tensor(out=ypr, in0=pzg,
                                               scalar=rstd[:, 1:2], in1=phis,
                                               op0=OP.mult, op1=OP.max)
                return (phis, ypr)

            def st_sort_a(pipe, iv, prev):
                phis, ypr = prev
                bufA = pipe.intermediate_tile([P, 2 * H], sort_t, name="bufA")
                bufB = pipe.intermediate_tile([P, 2 * H], sort_t, name="bufB")
                emit_sort_pass(0, phis, ypr, bufA)
                cur, nxt = bufA, bufB
                for pidx in range(1, split_pass):
                    if pidx < n_passes:
                        emit_sort_pass(pidx, cur, None, nxt)
                    cur, nxt = nxt, cur
                return (bufA, bufB)

            def st_sort_b(pipe, iv, prev):
                bufA, bufB = prev
                cur, nxt = (bufB, bufA) if split_pass % 2 == 0 else (bufA, bufB)
                for pidx in range(split_pass, 15):
                    if pidx < n_passes:
                        emit_sort_pass(pidx, cur, None, nxt)
                    cur, nxt = nxt, cur
                fin = cur  # pass 14 (even) -> bufA when split parity works out
                fv = fin.rearrange("p (h g e) -> p h g e", h=2, e=DPC)
                # coupling: u[i] <- max(u[i], v[i-1]) for i>=1, in place
                nc.vector.tensor_tensor(out=fv[:, 0, :, 1:DPC],
                                        in0=fv[:, 0, :, 1:DPC],
                                        in1=fv[:, 1, :, 0:DPC - 1], op=OP.max)
                red = sp.tile([P, 2, NCOMP], fp32, tag="red")
                nc.vector.tensor_reduce(out=red, in_=fv,
                                        axis=mybir.AxisListType.X, op=OP.add)
                comp = sp.tile([P, NCOMP], fp32, tag="comp")
                nc.vector.tensor_tensor(out=comp, in0=red[:, 1, :],
                                        in1=red[:, 0, :], op=OP.subtract)
                cs = sp.tile([P, 1], fp32, tag="cs")
                nc.vector.tensor_reduce(out=cs, in_=comp,
                                        axis=mybir.AxisListType.X, op=OP.add)
                cm = sp.tile([P, 1], fp32, tag="cm")
                nc.vector.tensor_reduce(out=cm, in_=comp,
                                        axis=mybir.AxisListType.X, op=OP.max)
                res = sp.tile([P, 1], fp32, tag="res")
                nc.vector.tensor_scalar(out=res, in0=cs, scalar1=avec[:, 0:1],
                                        scalar2=None, op0=OP.mult)
                nc.vector.scalar_tensor_tensor(out=res, in0=cm,
                                               scalar=avec[:, 1:2], in1=res,
                                               op0=OP.mult, op1=OP.add)
                nc.sync.dma_start(out=out_v[iv], in_=res[:, 0:1])

            def st_l01(pipe, iv, prev):
                xt, gt = prev
                t0o = mp.tile([P, H], fp32, tag="t0o")
                t0g = mp.tile([P, H], fp32, tag="t0g")
                gelu_ln(matmul_from(xt, 0), t0o)
                gelu_ln(matmul_from(gt, 0), t0g)
                oo = pipe.intermediate_tile([P, H], fp32, name="to1")
                og = pipe.intermediate_tile([P, H], fp32, name="tg1")
                gelu_ln(matmul_from(t0o, 1), oo)
                gelu_ln(matmul_from(t0g, 1), og)
                return (oo, og)

            def st_l0(pipe, iv, prev):
                xt, gt = prev
                oo = pipe.intermediate_tile([P, H], adt, name="to0")
                og = pipe.intermediate_tile([P, H], adt, name="tg0")
                gelu_ln2(matmul_from(xt, 0), matmul_from(gt, 0), oo, og)
                return (oo, og)

            stages = [st_load, st_l0, mk_layer(1), mk_layer(2), st_l3,
                      st_sort_a, st_sort_b]

            def run_pipe():
                he = (mybir.EngineType.PE, mybir.EngineType.DVE,
                      mybir.EngineType.Activation, mybir.EngineType.SP,
                      mybir.EngineType.Pool) if hints else ()
                tc.For_i_pipelined(stages, 0, nt, 1, pool=pipe_pool,
                                   unroll=unroll, staged_num_bufs=stage_bufs,
                                   hint_engines=he)

            if repeats == 1:
                run_pipe()
            else:
                with tc.For_i(0, repeats, 1):
                    run_pipe()

    nc.finalize()
    return nc


# ---------------------------------------------------------------- host wrapper
def _prep_host(inputs):
    """Fold LN affine params into the following layer's weights; build avec."""
    f32 = np.float32
    W0 = np.asarray(inputs["W0"], f32)
    b0 = np.asarray(inputs["b0"], f32)
    w, b = [W0], [b0]
    for i in (0, 1, 2):
        s = np.asarray(inputs[f"ln{i}_s"], f32)
        t = np.asarray(inputs[f"ln{i}_b"], f32)
        Wn = np.asarray(inputs[("W1", "W2", "W3")[i]], f32)
        bn = np.asarray(inputs[("b1", "b2", "b3")[i]], f32)
        w.append(s[:, None] * Wn)
        b.append(bn + t @ Wn)
    bs = np.stack(b, 0)  # [4, 512]
    alpha = float(np.asarray(inputs["alpha"]))
    a = 1.0 / (1.0 + np.exp(-alpha))
    avec = np.empty((P, 2), f32)
    avec[:, 0] = a / NCOMP
    avec[:, 1] = 1.0 - a
    return w[0], w[1], w[2], w[3], bs.astype(f32), avec


def _probe_devices():
    """Poke every core with a tiny op; retries to shake off a stale
    NRT_EXEC_UNIT_UNRECOVERABLE state left by a previous process."""
    import jax
    import jax.numpy as jnp

    for attempt in range(3):
        try:
            for d in jax.devices()[:NCORES]:
                jnp.zeros((1,), jnp.float32, device=d).block_until_ready()
            return
        except Exception:
            if attempt == 2:
                raise


def run_on_device(inputs, rows_total=B, trace=False, repeats=1, **build_kw):
    """Shard, run on 8 cores, gather. Returns (out [rows_total], results obj)."""
    from concourse.bass_utils import run_bass_kernel_spmd

    _probe_devices()

    rows_core = rows_total // NCORES
    key = (rows_core, repeats, tuple(sorted(build_kw.items())))
    if key not in _CACHE:
        _CACHE[key] = build_nc(rows_core, repeats=repeats, **build_kw)
    nc = _CACHE[key]

    w0, w1, w2, w3, bs, avec = _prep_host(inputs)
    onesv = np.ones((1, P), np.float32)
    if build_kw.get("act_dt") == "bf16":
        import ml_dtypes
        bf = ml_dtypes.bfloat16
        w0, w1, w2, w3 = (a.astype(bf) for a in (w0, w1, w2, w3))
        bs = bs.astype(bf)
        onesv = onesv.astype(bf)
    ob = np.ascontiguousarray(np.asarray(inputs["observations"], np.float32)[:rows_total])
    gl = np.ascontiguousarray(np.asarray(inputs["goals"], np.float32)[:rows_total])
    in_maps = []
    for c in range(NCORES):
        sl = slice(c * rows_core, (c + 1) * rows_core)
        in_maps.append({
            "observations": ob[sl], "goals": gl[sl],
            "w0": w0, "w1": w1, "w2": w2, "w3": w3, "bs": bs, "avec": avec,
            "onesv": onesv,
        })
    r = run_bass_kernel_spmd(nc, in_maps, list(range(NCORES)), trace=trace)
    outp = np.concatenate([r.results[c]["out"] for c in range(NCORES)])
    return outp, r


def kernel(**inputs):
    out, _ = run_on_device(inputs)
    return out.astype(np.float32)

